# revision 1
# baseline (speedup 1.0000x reference)
"""Causal single-head attention (B=4, S=4096, E=2048, H=128) on 8 trn2 cores.

Sharding: 2 cores per batch. Q rows split into four 1024-row chunks; core
group A takes chunks {0,3}, group B takes {1,2} (both causal-balanced at 72
k-tiles per core). Two SPMD programs run concurrently on disjoint 4-device
jax meshes (A on devices 0-3, B on 4-7).

Per-core pipeline (single pass over host-pretransposed x^T):
  1. Projections, W stationary: K^T,V^T [H, tok] for all 4096 tokens, Q^T
     for this core's 2048. fp32r matmuls accumulate 16 E-chunks in PSUM.
  2. V^T -> V [tok, H] via PE transposes (fp32, exact).
  3. Per 512-q block, per causal 128-k tile: scoresT = K_tile.T @ Q^T,
     exp via ACT (1/sqrt(H) folded into scale), diagonal-tile mask multiply,
     denominator accumulate (DVE), out^T += V_tile.T @ P^T accumulated in
     PSUM with the AV matmul emitted 2 iterations behind so PE never waits
     on ACT. Denominator broadcast = all-ones matmul; reciprocal; multiply;
     PE-transpose back to [q, H]; DMA out.
"""

import os
from contextlib import ExitStack

import numpy as np

import concourse.bacc as bacc
import concourse.bass as bass
import concourse.tile as tile
from concourse import mybir
from concourse.masks import make_identity

B, S, E, H = 4, 4096, 2048, 128
NE = E // 128            # 16 contraction chunks
NT = S // 512            # 8 tok chunks
QBLK = 512
KTILE = 128
SCALE = 1.0 / np.sqrt(H)

f32 = mybir.dt.float32
f32r = mybir.dt.float32r
AF = mybir.ActivationFunctionType

GROUP_CHUNKS = {0: (0, 3), 1: (1, 2)}


def _build_program(chunks, dt_mm):
    nc = bacc.Bacc("TRN2", target_bir_lowering=False, debug=False, num_devices=4)

    xT = nc.dram_tensor("xT", [E, S], dt_mm, kind="ExternalInput")
    ws = {k: nc.dram_tensor(f"w{k}", [E, H], dt_mm, kind="ExternalInput")
          for k in ("q", "k", "v")}
    bs = {k: nc.dram_tensor(f"b{k}", [H, 1], f32, kind="ExternalInput")
          for k in ("q", "k", "v")}
    masks_d = nc.dram_tensor("masks", [4 * 128, QBLK], dt_mm, kind="ExternalInput")
    out_d = nc.dram_tensor("out", [2048, H], f32, kind="ExternalOutput")

    qblocks = sorted([c * 1024 for c in chunks] + [c * 1024 + 512 for c in chunks])
    my_chunks = sorted({qb // 512 for qb in qblocks})
    qt_index = {t: i for i, t in enumerate(my_chunks)}

    with tile.TileContext(nc) as tc, ExitStack() as ctx:
        consts = ctx.enter_context(tc.tile_pool(name="consts", bufs=1))
        xt_pool = ctx.enter_context(tc.tile_pool(name="xt", bufs=2))
        kt_pool = ctx.enter_context(tc.tile_pool(name="kt", bufs=1))
        vt_pool = ctx.enter_context(tc.tile_pool(name="vtst", bufs=2))
        v_pool = ctx.enter_context(tc.tile_pool(name="v", bufs=1))
        qt_pool = ctx.enter_context(tc.tile_pool(name="qt", bufs=1))
        pt_pool = ctx.enter_context(tc.tile_pool(name="pt", bufs=4))
        den_pool = ctx.enter_context(tc.tile_pool(name="den", bufs=2))
        outn_pool = ctx.enter_context(tc.tile_pool(name="outn", bufs=2))
        outf_pool = ctx.enter_context(tc.tile_pool(name="outf", bufs=4))

        ps_mm = ctx.enter_context(tc.tile_pool(name="ps_mm", bufs=3, space="PSUM"))
        ps_tp = ctx.enter_context(tc.tile_pool(name="ps_tp", bufs=2, space="PSUM"))
        ps_out = ctx.enter_context(tc.tile_pool(name="ps_out", bufs=2, space="PSUM"))
        ps_den = ctx.enter_context(tc.tile_pool(name="ps_den", bufs=1, space="PSUM"))

        # ---- constants ----
        w_sb = {}
        for k in ("q", "k", "v"):
            w_sb[k] = consts.tile([128, NE, H], dt_mm, name=f"w_{k}", tag=f"w{k}")
            nc.sync.dma_start(
                out=w_sb[k], in_=ws[k].ap().rearrange("(n p) h -> p n h", p=128)
            )
        b_sb = {}
        for k in ("q", "k", "v"):
            b_sb[k] = consts.tile([H, 1], f32, name=f"b_{k}", tag=f"b{k}")
            nc.sync.dma_start(out=b_sb[k], in_=bs[k][:, :])
        masks_sb = consts.tile([128, 4, QBLK], dt_mm, tag="masks")
        nc.sync.dma_start(
            out=masks_sb, in_=masks_d.ap().rearrange("(j p) q -> p j q", p=128)
        )
        ident_f = consts.tile([128, 128], f32, tag="identf")
        make_identity(nc, ident_f)
        ones_mat = consts.tile([128, 128], f32, tag="ones")
        nc.vector.memset(ones_mat, 1.0)

        # ---- persistent on-chip tensors ----
        kt_tiles = [kt_pool.tile([H, 512], dt_mm, name=f"ktt{t}", tag=f"kt{t}") for t in range(NT)]
        v_tiles = [v_pool.tile([128, H], dt_mm, name=f"vt{j}", tag=f"v{j}") for j in range(S // 128)]
        qt_tiles = [qt_pool.tile([H, 512], dt_mm, name=f"qtt{i}", tag=f"qt{i}")
                    for i in range(len(my_chunks))]

        # ---- phase 1: projections ----
        for t in range(NT):
            xt = xt_pool.tile([128, NE, 512], dt_mm, tag="xt")
            src = xT.ap()[:, t * 512:(t + 1) * 512]
            nc.sync.dma_start(out=xt, in_=src.rearrange("(n p) s -> p n s", p=128))

            pk = ps_mm.tile([H, 512], f32, tag="mm")
            for e in range(NE):
                nc.tensor.matmul(pk, w_sb["k"][:, e, :], xt[:, e, :],
                                 start=(e == 0), stop=(e == NE - 1))
            nc.vector.tensor_scalar_add(kt_tiles[t][:, :], pk, b_sb["k"])

            pv = ps_mm.tile([H, 512], f32, tag="mm")
            for e in range(NE):
                nc.tensor.matmul(pv, w_sb["v"][:, e, :], xt[:, e, :],
                                 start=(e == 0), stop=(e == NE - 1))
            vt_sb = vt_pool.tile([H, 512], f32, tag="vt")
            nc.vector.tensor_scalar_add(vt_sb, pv, b_sb["v"])
            for j in range(4):
                ptp = ps_tp.tile([128, H], f32, tag="tp")
                nc.tensor.transpose(ptp, vt_sb[:, j * 128:(j + 1) * 128], ident_f)
                nc.scalar.copy(v_tiles[t * 4 + j][:, :], ptp)

            if t in qt_index:
                pq = ps_mm.tile([H, 512], f32, tag="mm")
                for e in range(NE):
                    nc.tensor.matmul(pq, w_sb["q"][:, e, :], xt[:, e, :],
                                     start=(e == 0), stop=(e == NE - 1))
                nc.vector.tensor_scalar_add(qt_tiles[qt_index[t]][:, :], pq,
                                            b_sb["q"])

        # ---- phase 2: attention ----
        for bi, qg in enumerate(qblocks):
            nk = qg // KTILE + 4
            qt = qt_tiles[qt_index[qg // 512]]

            po = ps_out.tile([H, QBLK], f32, tag="out")
            den = den_pool.tile([128, QBLK], f32, tag="den")
            pts = {}

            def emit_av(kt):
                nc.tensor.matmul(po, v_tiles[kt][:, :], pts.pop(kt),
                                 start=(kt == 0), stop=(kt == nk - 1))

            for kt in range(nk):
                st = ps_mm.tile([128, QBLK], f32, tag="mm")
                c, j = kt // 4, kt % 4
                nc.tensor.matmul(st, kt_tiles[c][:, j * 128:(j + 1) * 128],
                                 qt[:, :], start=True, stop=True)
                pt = pt_pool.tile([128, QBLK], dt_mm, tag="pt")
                nc.scalar.activation(pt, st, AF.Exp, scale=float(SCALE))
                if kt >= nk - 4:
                    nc.vector.tensor_mul(pt, pt, masks_sb[:, kt - (nk - 4), :])
                if kt == 0:
                    nc.vector.tensor_copy(den, pt)
                else:
                    nc.vector.tensor_add(den, den, pt)
                pts[kt] = pt
                if kt >= 2:
                    emit_av(kt - 2)
            emit_av(nk - 2)
            emit_av(nk - 1)

            pden = ps_den.tile([128, QBLK], f32, tag="pden")
            nc.tensor.matmul(pden, ones_mat[:, :], den, start=True, stop=True)
            recb = outn_pool.tile([128, QBLK], f32, tag="recb")
            nc.vector.reciprocal(recb, pden)

            outn = outn_pool.tile([128, QBLK], f32, tag="outn")
            nc.vector.tensor_mul(outn, po, recb)
            for j in range(4):
                ptp = ps_tp.tile([128, 128], f32, tag="tp")
                nc.tensor.transpose(ptp, outn[:, j * 128:(j + 1) * 128], ident_f)
                of = outf_pool.tile([128, H], f32, tag="of")
                nc.scalar.copy(of, ptp)
                row0 = bi * QBLK + j * 128
                nc.sync.dma_start(out=out_d.ap()[row0:row0 + 128, :], in_=of)

    nc.compile()
    return nc


_PROGRAMS = {}


def _get_program(group, dt_key):
    key = (group, dt_key)
    if key not in _PROGRAMS:
        dt_mm = f32r if dt_key == "f32r" else f32
        _PROGRAMS[key] = _build_program(GROUP_CHUNKS[group], dt_mm)
    return _PROGRAMS[key]


_FNS = {}


def _get_fn(nc, devices):
    """Build (once) and cache the jitted shard_map runner for `nc` on
    `devices`. Returns (fn, in_names, out_names, zero_outs)."""
    key = id(nc)
    if key in _FNS:
        return _FNS[key]
    import jax
    from jax.sharding import Mesh, PartitionSpec
    from jax.experimental.shard_map import shard_map
    from concourse.bass2jax import (_bass_exec_p, install_neuronx_cc_hook,
                                    partition_id_tensor)
    from concourse import mybir as _mybir

    install_neuronx_cc_hook()
    n_cores = len(devices)
    partition_name = (nc.partition_id_tensor.name
                      if nc.partition_id_tensor else None)

    in_names, out_names, out_avals, zero_outs = [], [], [], []
    for alloc in nc.m.functions[0].allocations:
        if not isinstance(alloc, _mybir.MemoryLocationSet):
            continue
        name = alloc.memorylocations[0].name
        if alloc.kind == "ExternalInput":
            if name != partition_name:
                in_names.append(name)
        elif alloc.kind == "ExternalOutput":
            shape = tuple(alloc.tensor_shape)
            dtype = _mybir.dt.np(alloc.dtype)
            out_names.append(name)
            out_avals.append(jax.core.ShapedArray(shape, dtype))
            zero_outs.append(np.zeros(shape, dtype))
    n_params = len(in_names)
    n_outs = len(out_avals)
    in_names_all = in_names + out_names
    if partition_name is not None:
        in_names_all = in_names_all + [partition_name]

    donate = tuple(range(n_params, n_params + n_outs))

    def _body(*args):
        operands = list(args)
        if partition_name is not None:
            operands.append(partition_id_tensor())
        outs = _bass_exec_p.bind(
            *operands,
            out_avals=tuple(out_avals),
            in_names=tuple(in_names_all),
            out_names=tuple(out_names),
            lowering_input_output_aliases=(),
            sim_require_finite=True,
            sim_require_nnan=True,
            nc=nc,
        )
        return tuple(outs)

    mesh = Mesh(np.asarray(devices), ("core",))
    in_specs = (PartitionSpec("core"),) * (n_params + n_outs)
    out_specs = (PartitionSpec("core"),) * n_outs
    fn = jax.jit(
        shard_map(_body, mesh=mesh, in_specs=in_specs, out_specs=out_specs,
                  check_rep=False),
        donate_argnums=donate, keep_unused=True,
    )
    _FNS[key] = (fn, in_names, out_names, zero_outs)
    return _FNS[key]


def _run_on_devices(nc, in_maps, devices):
    fn, in_names, out_names, zero_outs = _get_fn(nc, devices)
    n_cores = len(devices)
    per_core = [[np.asarray(m[name]) for name in in_names] for m in in_maps]
    concat_in = [np.concatenate([per_core[c][i] for c in range(n_cores)], axis=0)
                 for i in range(len(in_names))]
    concat_zeros = [np.zeros((n_cores * z.shape[0], *z.shape[1:]), z.dtype)
                    for z in zero_outs]
    out_arrs = fn(*concat_in, *concat_zeros)
    return out_arrs, out_names


def _make_masks():
    m = np.zeros((4, 128, QBLK), dtype=np.float32)
    kk = np.arange(128)[:, None]
    qq = np.arange(QBLK)[None, :]
    for j in range(4):
        m[j] = ((128 * j + kk) <= qq).astype(np.float32)
    return np.ascontiguousarray(m.reshape(4 * 128, QBLK))


def _prep_in_maps(x, Wq_w, Wq_b, Wk_w, Wk_b, Wv_w, Wv_b):
    masks = _make_masks()
    common_w = {
        "wq": np.ascontiguousarray(Wq_w, dtype=np.float32),
        "wk": np.ascontiguousarray(Wk_w, dtype=np.float32),
        "wv": np.ascontiguousarray(Wv_w, dtype=np.float32),
        "bq": np.ascontiguousarray(Wq_b, dtype=np.float32).reshape(H, 1),
        "bk": np.ascontiguousarray(Wk_b, dtype=np.float32).reshape(H, 1),
        "bv": np.ascontiguousarray(Wv_b, dtype=np.float32).reshape(H, 1),
        "masks": masks,
    }
    maps = {0: [], 1: []}
    for b in range(B):
        xTb = np.ascontiguousarray(np.asarray(x[b], dtype=np.float32).T)
        maps[0].append({"xT": xTb, **common_w})
        maps[1].append({"xT": xTb, **common_w})
    return maps


def kernel(x, Wq_w, Wq_b, Wk_w, Wk_b, Wv_w, Wv_b):
    import jax

    dt_key = os.environ.get("ATTN_MM_DTYPE", "f32r")
    ncA = _get_program(0, dt_key)
    ncB = _get_program(1, dt_key)

    maps = _prep_in_maps(x, Wq_w, Wq_b, Wk_w, Wk_b, Wv_w, Wv_b)
    devs = jax.devices()
    outA, namesA = _run_on_devices(ncA, maps[0], devs[0:4])
    outB, namesB = _run_on_devices(ncB, maps[1], devs[4:8])

    oA = np.asarray(outA[namesA.index("out")]).reshape(4, 2048, H)
    oB = np.asarray(outB[namesB.index("out")]).reshape(4, 2048, H)

    out = np.empty((B, S, H), dtype=np.float32)
    for b in range(B):
        out[b, 0:1024] = oA[b, 0:1024]
        out[b, 3072:4096] = oA[b, 1024:2048]
        out[b, 1024:2048] = oB[b, 0:1024]
        out[b, 2048:3072] = oB[b, 1024:2048]
    return out

